# revision 9
# baseline (speedup 1.0000x reference)
"""Cross-attention kernel for Trainium2, sharded over 8 NeuronCores.

Sharding: core c handles batch b = c // 4 and head-group g = c % 4
(4 of 16 heads, i.e. 256 of 1024 channels). Each core computes
  q_g = query[b] @ Wq[g].T ; k_g = key[b] @ Wk[g].T ; v_g = value[b] @ Wv[g].T
  x_g = softmax(q_g k_g^T * scale) v_g          (4 heads, independent)
  partial_g = x_g @ Wp[:, g].T                  (partial over head-group)
Host sums the 4 (bf16) partials per batch and adds the bias terms
(bp exactly, and bv folded through: softmax rows sum to 1, so the value
bias contributes bv @ Wp.T to every token).

Schedule: the softmax exp on the Scalar engine (1 elem/cycle/lane) is the
hard bottleneck (~16.8M exps/core), so the attention m-loop is organized
around back-to-back ACT instructions, and the projection matmuls that
don't gate the first q-chunk are injected one-per-step into the attention
m-loops to fill the PE's idle slots: q-chunk c+1 is projected during
chunk c's first head-pair loop, and chunk c-1's output projection runs
during chunk c's second head-pair loop. All matmul operands are bf16 and
all bulk inputs are pre-tiled on the host into SBUF layout so every DMA
is a cheap 2D descriptor issued in priority order. Scores are computed
transposed (scoresT[m, q]) with two heads row-packed per matmul via
tile_position, the softmax denominator comes from a ones-augmented value
matrix, and PV chains directly off the exp output. The per-head
normalization (reciprocal of the denominator broadcast across
partitions) runs on Vector + GpSimd only, so it never blocks the
in-order PE queue; the PV accumulator bank is released by short
psum->sbuf copies.

PSUM budget (8 banks): scores 2x[128,1024] (4) + xtA/xtB accumulators (2)
+ a 2-slot shared pool (one tag) that q-proj and out-proj accumulators
rotate through (2).
"""

import numpy as np
import ml_dtypes

import concourse.bass as bass
import concourse.mybir as mybir
import concourse.tile as tile
from concourse import bacc
from concourse.bass_utils import run_bass_kernel_spmd

B, N, DIM, H, DH = 2, 2048, 1024, 16, 64
NCORES = 8
HG = 4            # head-groups (cores per batch)
HPG = H // HG     # heads per group = 4
CS = DIM // HG    # channels per group = 256
P = 128
KT = DIM // P     # 8 contraction tiles for the projections
NT = N // P       # 16 token tiles
QW = 512          # q-chunk width (moving-operand max free dim)
QC = N // QW      # 4 q-chunks

FP32 = mybir.dt.float32
BF16 = mybir.dt.bfloat16
AF = mybir.ActivationFunctionType
BF16NP = ml_dtypes.bfloat16


def _build(scale: float, add_qk_bias: bool, reps: int = 1,
           loop_reps: int | None = None):
    nc = bacc.Bacc("TRN2", target_bir_lowering=False, debug=False,
                   num_devices=NCORES)

    # All host-side pre-tiled to SBUF layout: partition-major, k-tile
    # blocked along the free dim, so every DMA is a plain 2D copy.
    qT = nc.dram_tensor("qT", [P, QC * KT * QW], BF16,
                        kind="ExternalInput").ap()   # chunk-blocked
    kT = nc.dram_tensor("kT", [P, KT * N], BF16, kind="ExternalInput").ap()
    vT = nc.dram_tensor("vT", [P, KT * N], BF16, kind="ExternalInput").ap()
    wq = nc.dram_tensor("wq", [P, KT * CS], BF16, kind="ExternalInput").ap()
    wk = nc.dram_tensor("wk", [P, KT * CS], BF16, kind="ExternalInput").ap()
    wv = nc.dram_tensor("wv", [P, KT * CS], BF16, kind="ExternalInput").ap()
    wp = nc.dram_tensor("wp", [P, (CS // P) * DIM], BF16,
                        kind="ExternalInput").ap()
    bqk = nc.dram_tensor("bqk", [P, 2 * (CS // P)], FP32,
                         kind="ExternalInput").ap()
    out = nc.dram_tensor("out", [DIM, N], BF16, kind="ExternalOutput").ap()

    from contextlib import ExitStack, nullcontext
    with nc.allow_low_precision(reason="bf16 matmul rounding is intended"), \
         tile.TileContext(nc) as tc, ExitStack() as stack:
        wpool = stack.enter_context(tc.tile_pool(name="wpool", bufs=1))
        persist = stack.enter_context(tc.tile_pool(name="persist", bufs=1))
        const = stack.enter_context(tc.tile_pool(name="const", bufs=1))

        wk_sb = wpool.tile([P, KT * CS], BF16, tag="wk")
        wv_sb = wpool.tile([P, KT * CS], BF16, tag="wv")
        wq_sb = wpool.tile([P, KT * CS], BF16, tag="wq")
        wp_sb = wpool.tile([P, (CS // P) * DIM], BF16, tag="wp")
        bqk_sb = const.tile([P, 2 * (CS // P)], FP32, tag="bqk")
        warm = const.tile([1, 8], FP32, tag="warm")
        nc.vector.memset(warm[:], 0.0)
        nc.scalar.activation(warm[:], warm[:], AF.Exp, scale=1.0)

        # Persistent activations (bf16).
        qsb = [[persist.tile([P, QW], BF16, tag=f"qsb{t}_{c}",
                             name=f"qsb{t}_{c}")
                for c in range(QC)] for t in range(2)]
        ksb = [persist.tile([P, N], BF16, tag=f"ksb{t}", name=f"ksb{t}")
               for t in range(2)]
        # v token-major with a ones column per head: [tok, 4*(64+1)]
        vsb = [persist.tile([P, HPG * (DH + 1)], BF16, tag=f"vsb{t}",
                            name=f"vsb{t}")
               for t in range(NT)]
        # resident inputs; one tile per DMA so every write is whole-tile
        qres0 = persist.tile([P, KT * QW], BF16, tag="qres0")
        qres123 = persist.tile([P, (QC - 1) * KT * QW], BF16, tag="qres123")
        vres_a = persist.tile([P, KT // 2 * N], BF16, tag="vres_a")
        vres_b = persist.tile([P, KT // 2 * N], BF16, tag="vres_b")

        def qres_mv(c, k):
            """Moving operand for q-proj chunk c, k-tile k."""
            if c == 0:
                return qres0[:, k * QW:(k + 1) * QW]
            off = (c - 1) * KT * QW + k * QW
            return qres123[:, off:off + QW]

        def vres_st(k, tt):
            """Stationary operand for v-proj: k-tile k, token-tile tt."""
            t_ = vres_a if k < KT // 2 else vres_b
            off = (k % (KT // 2)) * N + tt * P
            return t_[:, off:off + P]

        loop_cm = (tc.For_i(0, loop_reps, 1) if loop_reps else nullcontext())
        with loop_cm:
          for rep in range(reps):
            rn = f"r{rep}"
            with tc.tile_pool(name="kstream", bufs=KT) as kstream:
                # ---- bulk input DMAs, priority order ------------------
                nc.sync.dma_start(out=wk_sb[:], in_=wk[:])
                nc.sync.dma_start(out=wv_sb[:], in_=wv[:])
                nc.sync.dma_start(out=wq_sb[:], in_=wq[:])
                if add_qk_bias:
                    nc.sync.dma_start(out=bqk_sb[:], in_=bqk[:])
                kts = []
                for k in range(KT):
                    ts_ = kstream.tile([P, N], BF16, tag="ks",
                                       name=f"ks{k}_{rn}")
                    nc.sync.dma_start(out=ts_[:],
                                      in_=kT[:, k * N:(k + 1) * N])
                    kts.append(ts_)
                    if k == 2:
                        nc.sync.dma_start(out=qres0[:], in_=qT[:, 0:KT * QW])
                nc.sync.dma_start(out=vres_a[:], in_=vT[:, 0:KT // 2 * N])
                nc.sync.dma_start(out=vres_b[:], in_=vT[:, KT // 2 * N:])
                nc.sync.dma_start(out=qres123[:], in_=qT[:, KT * QW:])
                nc.sync.dma_start(out=wp_sb[:], in_=wp[:])

                # ---- K projection (channel-major) ---------------------
                with tc.tile_pool(name="kpsum", bufs=1,
                                  space="PSUM") as kpp:
                    pA = kpp.tile([P, N], FP32, tag="pA")
                    pB = kpp.tile([P, N], FP32, tag="pB")
                    for k in range(KT):
                        for nn2 in range(QC):
                            nc.tensor.matmul(
                                pA[:, nn2 * QW:(nn2 + 1) * QW],
                                wk_sb[:, k * CS:k * CS + P],
                                kts[k][:, nn2 * QW:(nn2 + 1) * QW],
                                start=(k == 0), stop=(k == KT - 1))
                            nc.tensor.matmul(
                                pB[:, nn2 * QW:(nn2 + 1) * QW],
                                wk_sb[:, k * CS + P:(k + 1) * CS],
                                kts[k][:, nn2 * QW:(nn2 + 1) * QW],
                                start=(k == 0), stop=(k == KT - 1))
                    for t, pt in enumerate((pA, pB)):
                        for cc in range(QC):
                            cs_ = slice(cc * QW, (cc + 1) * QW)
                            if add_qk_bias:
                                nc.vector.tensor_scalar(
                                    ksb[t][:, cs_], pt[:, cs_],
                                    bqk_sb[:, CS // P + t:CS // P + t + 1],
                                    None, mybir.AluOpType.add)
                            else:
                                nc.vector.tensor_copy(ksb[t][:, cs_],
                                                      pt[:, cs_])

            # ---- Q projection, chunk 0 only (eager) --------------------
            with tc.tile_pool(name="q0psum", bufs=2, space="PSUM") as q0p:
                for step in _qproj_steps(nc, tc, q0p, wq_sb, qres_mv, qsb,
                                         bqk_sb, add_qk_bias, 0, rn):
                    step()

            # ---- V projection (token-major) ----------------------------
            with tc.tile_pool(name="pv", bufs=4, space="PSUM") as pvp:
                for tt in range(NT):
                    pvt = pvp.tile([P, CS], FP32, tag="pv",
                                   name=f"pv{tt}_{rn}")
                    for k in range(KT):
                        nc.tensor.matmul(
                            pvt[:],
                            vres_st(k, tt),
                            wv_sb[:, k * CS:(k + 1) * CS],
                            start=(k == 0), stop=(k == KT - 1))
                    dst3 = vsb[tt][:].rearrange("p (h c) -> p h c", h=HPG)
                    nc.vector.tensor_copy(
                        dst3[:, :, 0:DH],
                        pvt[:].rearrange("p (h c) -> p h c", h=HPG))
                    nc.vector.memset(
                        dst3[:, :, DH:DH + 1].bitcast(mybir.dt.uint16),
                        0x3f80)

            # ---- Attention + injected projections, per q-chunk ---------
            with tc.tile_pool(name="probs", bufs=3) as probs, \
                 tc.tile_pool(name="xq", bufs=2) as xqp, \
                 tc.tile_pool(name="xs", bufs=2) as xsp, \
                 tc.tile_pool(name="small", bufs=2) as small, \
                 tc.tile_pool(name="ost", bufs=4) as ostp, \
                 tc.tile_pool(name="psc", bufs=2, space="PSUM") as psc, \
                 tc.tile_pool(name="pxt", bufs=1, space="PSUM") as pxt, \
                 tc.tile_pool(name="shpp", bufs=2, space="PSUM") as shp:
                xq_of = {}
                for qq in range(QC):
                    xq = [xqp.tile([P, QW], BF16, tag=f"x{t}",
                                   name=f"xq{t}_{qq}_{rn}") for t in range(2)]
                    xq_of[qq] = xq
                    inj0 = (_qproj_steps(nc, tc, shp, wq_sb, qres_mv, qsb,
                                         bqk_sb, add_qk_bias, qq + 1, rn)
                            if qq + 1 < QC else [])
                    inj1 = (_oproj_steps(nc, shp, ostp, wp_sb,
                                         xq_of[qq - 1], out, qq - 1, rn,
                                         tc=tc)
                            if qq > 0 else [])
                    for hp in range(HPG // 2):
                        inj = inj0 if hp == 0 else inj1
                        pt = hp
                        xtA = pxt.tile([P, QW], FP32, tag="xtA")
                        xtB = pxt.tile([P, QW], FP32, tag="xtB")
                        for m in range(NT):
                            sc = psc.tile([P, 2 * QW], FP32, tag="sc")
                            pr = probs.tile([P, 2 * QW], BF16, tag="pr")
                            for j, off in ((0, 0), (1, DH)):
                                nc.tensor.matmul(
                                    sc[:, j * QW:(j + 1) * QW],
                                    ksb[pt][off:off + DH, m * P:(m + 1) * P],
                                    qsb[pt][qq][off:off + DH, :],
                                    start=True, stop=True,
                                    tile_position=(off, 0))
                            nc.scalar.activation(pr[:], sc[:], AF.Exp,
                                                 scale=scale)
                            for j, xt, h in ((0, xtA, 2 * hp),
                                             (1, xtB, 2 * hp + 1)):
                                nc.tensor.matmul(
                                    xt[0:DH + 1, :],
                                    vsb[m][:, h * (DH + 1):(h + 1) * (DH + 1)],
                                    pr[:, j * QW:(j + 1) * QW],
                                    start=(m == 0), stop=(m == NT - 1))
                            if m < len(inj):
                                inj[m]()
                        for step in inj[NT:]:
                            step()
                        # normalization: vector + gpsimd only (PE-free);
                        # short copies release the psum bank.
                        den2 = small.tile([1, 2 * QW], FP32, tag="den2")
                        xs2 = xsp.tile([DH, 2 * QW], FP32, tag="xs2",
                                       name=f"xs2_{qq}_{hp}_{rn}")
                        for xt, j in ((xtA, 0), (xtB, 1)):
                            nc.vector.tensor_copy(
                                den2[:, j * QW:(j + 1) * QW],
                                xt[DH:DH + 1, :])
                        rde = small.tile([1, 2 * QW], FP32, tag="rde")
                        nc.vector.reciprocal_approx_fast(out=rde[:],
                                                         in_=den2[:])
                        bcg = small.tile([DH, 2 * QW], FP32, tag="bcg")
                        nc.gpsimd.partition_broadcast(bcg[:], rde[:],
                                                      channels=DH)
                        for xt, j in ((xtA, 0), (xtB, 1)):
                            nc.vector.tensor_copy(
                                xs2[:, j * QW:(j + 1) * QW], xt[0:DH, :])
                        for j, off in ((0, 0), (1, DH)):
                            nc.vector.tensor_mul(
                                xq[pt][off:off + DH, :],
                                xs2[:, j * QW:(j + 1) * QW],
                                bcg[:, j * QW:(j + 1) * QW])
                # tail: output projection for the last q-chunk
                for step in _oproj_steps(nc, shp, ostp, wp_sb,
                                         xq_of[QC - 1], out, QC - 1, rn):
                    step()

    nc.compile()
    return nc


def _qproj_steps(nc, tc, pool, wq_sb, qres_mv, qsb, bqk_sb, add_qk_bias,
                 c, rn):
    """Q-projection of chunk c as a list of closures, ONE matmul per
    attention m-step (16 steps), then the psum->sbuf copies."""
    state = {}

    def mk_mm(k, t):
        def step():
            if k == 0 and t == 0:
                state["pA"] = pool.tile([P, QW], FP32, tag="sh",
                                        name=f"pjA{c}_{rn}")
                state["pB"] = pool.tile([P, QW], FP32, tag="sh",
                                        name=f"pjB{c}_{rn}")
            dst = state["pA"] if t == 0 else state["pB"]
            w = (wq_sb[:, k * CS:k * CS + P] if t == 0
                 else wq_sb[:, k * CS + P:(k + 1) * CS])
            nc.tensor.matmul(dst[:], w, qres_mv(c, k),
                             start=(k == 0), stop=(k == KT - 1))
        return step

    def mk_copy(t):
        def step():
            pt = state["pA"] if t == 0 else state["pB"]
            with tc.high_priority():
                if add_qk_bias:
                    nc.vector.tensor_scalar(qsb[t][c][:], pt[:],
                                            bqk_sb[:, t:t + 1], None,
                                            mybir.AluOpType.add)
                else:
                    nc.vector.tensor_copy(qsb[t][c][:], pt[:])
        return step

    steps = []
    for k in range(KT):
        steps.append(mk_mm(k, 0))
        steps.append(mk_mm(k, 1))
    return steps + [mk_copy(0), mk_copy(1)]


def _oproj_steps(nc, pool, ostp, wp_sb, xq, out, qq, rn, tc=None):
    """Output projection of chunk qq as per-m-step closures: one mo-tile
    (two accumulating matmuls + copy + DMA out) every two steps."""
    qs = slice(qq * QW, (qq + 1) * QW)
    state = {}
    steps = []

    from contextlib import nullcontext

    def delay():
        return tc.high_priority(offset=-40) if tc is not None else nullcontext()

    def mk_mm(mo, k2):
        def step():
            with delay():
                if k2 == 0:
                    state[mo] = pool.tile([P, QW], FP32, tag="sh",
                                          name=f"po{mo}_{qq}_{rn}")
                nc.tensor.matmul(
                    state[mo][:],
                    wp_sb[:, k2 * DIM + mo * P:k2 * DIM + (mo + 1) * P],
                    xq[k2][:],
                    start=(k2 == 0), stop=(k2 == CS // P - 1))
        return step

    def mk_out(mo):
        def step():
            with delay():
                ost = ostp.tile([P, QW], BF16, tag="ost",
                                name=f"ost{mo}_{qq}_{rn}")
                nc.vector.tensor_copy(ost[:], state[mo][:])
                nc.sync.dma_start(out=out[mo * P:(mo + 1) * P, qs],
                                  in_=ost[:])
        return step

    for mo in range(KT):
        steps.append(mk_mm(mo, 0))

        def both(mo=mo):
            mk_mm(mo, 1)()
            mk_out(mo)()
        steps.append(both)
    return steps


_CACHE = {}


def _get_program(scale: float, add_qk_bias: bool, reps: int = 1,
                 loop_reps=None):
    key = (scale, add_qk_bias, reps, loop_reps)
    if key not in _CACHE:
        _CACHE[key] = _build(scale, add_qk_bias, reps, loop_reps)
    return _CACHE[key]


def _ktile(x, kt):
    """[kt*P, F] -> [P, kt*F] with k-tile blocks along the free dim."""
    f = x.shape[1]
    return np.ascontiguousarray(
        x.reshape(kt, P, f).transpose(1, 0, 2).reshape(P, kt * f))


def make_in_maps(query, key, value, Wq, bq, Wk, bk, Wv, bv, Wp, bp, scale):
    query = np.asarray(query, np.float32)
    key = np.asarray(key, np.float32)
    value = np.asarray(value, np.float32)
    Wq, Wk, Wv, Wp = (np.asarray(a, np.float32) for a in (Wq, Wk, Wv, Wp))
    bq, bk = np.asarray(bq, np.float32), np.asarray(bk, np.float32)
    in_maps = []
    for c in range(NCORES):
        b, g = c // HG, c % HG
        cs = slice(g * CS, (g + 1) * CS)
        bqk_arr = np.stack([bq[cs].reshape(CS // P, P),
                            bk[cs].reshape(CS // P, P)]).reshape(-1, P).T
        # qT chunk-blocked: [P, (c k n)] with c = q-chunk
        qt = query[b].T.reshape(KT, P, QC, QW).transpose(1, 2, 0, 3)
        in_maps.append({
            "qT": np.ascontiguousarray(qt.reshape(P, QC * KT * QW)
                                       ).astype(BF16NP),
            "kT": _ktile(key[b].T, KT).astype(BF16NP),
            "vT": _ktile(value[b].T, KT).astype(BF16NP),
            "wq": _ktile(np.ascontiguousarray(Wq[cs, :].T), KT).astype(BF16NP),
            "wk": _ktile(np.ascontiguousarray(Wk[cs, :].T), KT).astype(BF16NP),
            "wv": _ktile(np.ascontiguousarray(Wv[cs, :].T), KT).astype(BF16NP),
            "wp": _ktile(np.ascontiguousarray(Wp[:, cs].T),
                         CS // P).astype(BF16NP),
            "bqk": np.ascontiguousarray(bqk_arr),
        })
    return in_maps


def combine_outputs(results, bv, bp, Wp):
    bv = np.asarray(bv, np.float32)
    bp = np.asarray(bp, np.float32)
    Wp = np.asarray(Wp, np.float32)
    out = np.empty((B, N, DIM), np.float32)
    corr = bp + bv @ Wp.T
    for b in range(B):
        acc = results[b * HG]["out"].astype(np.float32)
        for g in range(1, HG):
            acc += results[b * HG + g]["out"].astype(np.float32)
        out[b] = acc.T + corr
    return out


def kernel(query, key, value, Wq, bq, Wk, bk, Wv, bv, Wp, bp, scale):
    scale_v = float(np.asarray(scale).reshape(-1)[0])
    add_qk_bias = bool(np.any(np.asarray(bq)) or np.any(np.asarray(bk)))
    nc = _get_program(scale_v, add_qk_bias)
    in_maps = make_in_maps(query, key, value, Wq, bq, Wk, bk, Wv, bv,
                           Wp, bp, scale)
    res = run_bass_kernel_spmd(nc, in_maps, list(range(NCORES))).results
    return combine_outputs(res, bv, bp, Wp)


# revision 11
# speedup vs baseline: 1.0664x; 1.0664x over previous
"""Cross-attention kernel for Trainium2, sharded over 8 NeuronCores.

Sharding: core c handles batch b = c // 4 and head-group g = c % 4
(4 of 16 heads, i.e. 256 of 1024 channels). Each core computes
  q_g = query[b] @ Wq[g].T ; k_g = key[b] @ Wk[g].T ; v_g = value[b] @ Wv[g].T
  x_g = softmax(q_g k_g^T * scale) v_g          (4 heads, independent)
  partial_g = x_g @ Wp[:, g].T                  (partial over head-group)
Host sums the 4 (bf16) partials per batch and adds the bias terms
(bp exactly, and bv folded through: softmax rows sum to 1, so the value
bias contributes bv @ Wp.T to every token).

Schedule: the softmax exp on the Scalar engine (1 elem/cycle/lane) is the
hard bottleneck (~16.8M exps/core), so the attention m-loop is organized
around back-to-back ACT instructions, and the projection matmuls that
don't gate the first q-chunk are injected one-per-step into the attention
m-loops to fill the PE's idle slots: q-chunk c+1 is projected during
chunk c's first head-pair loop, and chunk c-1's output projection runs
during chunk c's second head-pair loop. All matmul operands are bf16 and
all bulk inputs are pre-tiled on the host into SBUF layout so every DMA
is a cheap 2D descriptor issued in priority order. Scores are computed
transposed (scoresT[m, q]) with two heads row-packed per matmul via
tile_position, the softmax denominator comes from a ones-augmented value
matrix, and PV chains directly off the exp output. The per-head
normalization (reciprocal of the denominator broadcast across
partitions) runs on Vector + GpSimd only, so it never blocks the
in-order PE queue; the PV accumulator bank is released by short
psum->sbuf copies.

PSUM budget (8 banks): scores 2x[128,1024] (4) + xtA/xtB accumulators (2)
+ a 2-slot shared pool (one tag) that q-proj and out-proj accumulators
rotate through (2).
"""

import numpy as np
import ml_dtypes

import concourse.bass as bass
import concourse.mybir as mybir
import concourse.tile as tile
from concourse import bacc
from concourse.bass_utils import run_bass_kernel_spmd

B, N, DIM, H, DH = 2, 2048, 1024, 16, 64
NCORES = 8
HG = 4            # head-groups (cores per batch)
HPG = H // HG     # heads per group = 4
CS = DIM // HG    # channels per group = 256
P = 128
KT = DIM // P     # 8 contraction tiles for the projections
NT = N // P       # 16 token tiles
QW = 512          # q-chunk width (moving-operand max free dim)
QC = N // QW      # 4 q-chunks

FP32 = mybir.dt.float32
BF16 = mybir.dt.bfloat16
AF = mybir.ActivationFunctionType
BF16NP = ml_dtypes.bfloat16


def _build(scale: float, add_qk_bias: bool, reps: int = 1,
           loop_reps: int | None = None):
    nc = bacc.Bacc("TRN2", target_bir_lowering=False, debug=False,
                   num_devices=NCORES)

    # All host-side pre-tiled to SBUF layout: partition-major, k-tile
    # blocked along the free dim, so every DMA is a plain 2D copy.
    qT = nc.dram_tensor("qT", [P, QC * KT * QW], BF16,
                        kind="ExternalInput").ap()   # chunk-blocked
    kT = nc.dram_tensor("kT", [P, KT * N], BF16, kind="ExternalInput").ap()
    vT = nc.dram_tensor("vT", [P, KT * N], BF16, kind="ExternalInput").ap()
    wq = nc.dram_tensor("wq", [P, KT * CS], BF16, kind="ExternalInput").ap()
    wk = nc.dram_tensor("wk", [P, KT * CS], BF16, kind="ExternalInput").ap()
    wv = nc.dram_tensor("wv", [P, KT * CS], BF16, kind="ExternalInput").ap()
    wp = nc.dram_tensor("wp", [P, (CS // P) * DIM], BF16,
                        kind="ExternalInput").ap()
    bqk = nc.dram_tensor("bqk", [P, 2 * (CS // P)], FP32,
                         kind="ExternalInput").ap()
    out = nc.dram_tensor("out", [DIM, N], BF16, kind="ExternalOutput").ap()

    from contextlib import ExitStack, nullcontext
    with nc.allow_low_precision(reason="bf16 matmul rounding is intended"), \
         tile.TileContext(nc) as tc, ExitStack() as stack:
        wpool = stack.enter_context(tc.tile_pool(name="wpool", bufs=1))
        persist = stack.enter_context(tc.tile_pool(name="persist", bufs=1))
        const = stack.enter_context(tc.tile_pool(name="const", bufs=1))

        wk_sb = wpool.tile([P, KT * CS], BF16, tag="wk")
        wv_sb = wpool.tile([P, KT * CS], BF16, tag="wv")
        wq_sb = wpool.tile([P, KT * CS], BF16, tag="wq")
        wp_sb = wpool.tile([P, (CS // P) * DIM], BF16, tag="wp")
        bqk_sb = const.tile([P, 2 * (CS // P)], FP32, tag="bqk")
        warm = const.tile([1, 8], FP32, tag="warm")
        nc.vector.memset(warm[:], 0.0)
        nc.scalar.activation(warm[:], warm[:], AF.Exp, scale=1.0)

        # Persistent activations (bf16).
        qsb = [[persist.tile([P, QW], BF16, tag=f"qsb{t}_{c}",
                             name=f"qsb{t}_{c}")
                for c in range(QC)] for t in range(2)]
        ksb = [persist.tile([P, N], BF16, tag=f"ksb{t}", name=f"ksb{t}")
               for t in range(2)]
        # v token-major with a ones column per head: [tok, 4*(64+1)]
        vsb = [persist.tile([P, HPG * (DH + 1)], BF16, tag=f"vsb{t}",
                            name=f"vsb{t}")
               for t in range(NT)]
        # resident inputs; one tile per DMA so every write is whole-tile
        qres0 = persist.tile([P, KT * QW], BF16, tag="qres0")
        qres123 = persist.tile([P, (QC - 1) * KT * QW], BF16, tag="qres123")
        vres_a = persist.tile([P, KT // 2 * N], BF16, tag="vres_a")
        vres_b = persist.tile([P, KT // 2 * N], BF16, tag="vres_b")

        def qres_mv(c, k):
            """Moving operand for q-proj chunk c, k-tile k."""
            if c == 0:
                return qres0[:, k * QW:(k + 1) * QW]
            off = (c - 1) * KT * QW + k * QW
            return qres123[:, off:off + QW]

        def vres_st(k, tt):
            """Stationary operand for v-proj: k-tile k, token-tile tt."""
            t_ = vres_a if k < KT // 2 else vres_b
            off = (k % (KT // 2)) * N + tt * P
            return t_[:, off:off + P]

        loop_cm = (tc.For_i(0, loop_reps, 1) if loop_reps else nullcontext())
        with loop_cm:
          for rep in range(reps):
            rn = f"r{rep}"
            with tc.tile_pool(name="kstream", bufs=KT) as kstream:
                # ---- bulk input DMAs, priority order ------------------
                nc.sync.dma_start(out=wk_sb[:], in_=wk[:])
                nc.sync.dma_start(out=wv_sb[:], in_=wv[:])
                nc.sync.dma_start(out=wq_sb[:], in_=wq[:])
                if add_qk_bias:
                    nc.sync.dma_start(out=bqk_sb[:], in_=bqk[:])
                kts = []
                for k in range(KT):
                    ts_ = kstream.tile([P, N], BF16, tag="ks",
                                       name=f"ks{k}_{rn}")
                    nc.sync.dma_start(out=ts_[:],
                                      in_=kT[:, k * N:(k + 1) * N])
                    kts.append(ts_)
                    if k == 2:
                        nc.sync.dma_start(out=qres0[:], in_=qT[:, 0:KT * QW])
                nc.sync.dma_start(out=vres_a[:], in_=vT[:, 0:KT // 2 * N])
                nc.sync.dma_start(out=vres_b[:], in_=vT[:, KT // 2 * N:])
                nc.sync.dma_start(out=qres123[:], in_=qT[:, KT * QW:])
                nc.sync.dma_start(out=wp_sb[:], in_=wp[:])

                # ---- K projection (channel-major) ---------------------
                with tc.tile_pool(name="kpsum", bufs=1,
                                  space="PSUM") as kpp:
                    pA = kpp.tile([P, N], FP32, tag="pA")
                    pB = kpp.tile([P, N], FP32, tag="pB")
                    for k in range(KT):
                        for nn2 in range(QC):
                            nc.tensor.matmul(
                                pA[:, nn2 * QW:(nn2 + 1) * QW],
                                wk_sb[:, k * CS:k * CS + P],
                                kts[k][:, nn2 * QW:(nn2 + 1) * QW],
                                start=(k == 0), stop=(k == KT - 1))
                            nc.tensor.matmul(
                                pB[:, nn2 * QW:(nn2 + 1) * QW],
                                wk_sb[:, k * CS + P:(k + 1) * CS],
                                kts[k][:, nn2 * QW:(nn2 + 1) * QW],
                                start=(k == 0), stop=(k == KT - 1))
                    for t, pt in enumerate((pA, pB)):
                        for cc in range(QC):
                            cs_ = slice(cc * QW, (cc + 1) * QW)
                            if add_qk_bias:
                                nc.vector.tensor_scalar(
                                    ksb[t][:, cs_], pt[:, cs_],
                                    bqk_sb[:, CS // P + t:CS // P + t + 1],
                                    None, mybir.AluOpType.add)
                            else:
                                nc.vector.tensor_copy(ksb[t][:, cs_],
                                                      pt[:, cs_])

            # ---- Q projection chunk 0 + V-proj tiles 0-1 (eager), then
            # ---- attention with injected projections, per q-chunk.
            # Injection map: qq0-hp0: V-proj tiles 2..15; qq0-hp1: q-chunk1;
            # qq1-hp0: q-chunk2; qq1-hp1: oproj0; qq2-hp0: q-chunk3;
            # qq2-hp1: oproj1; qq3-hp0: oproj2; tail: oproj3.
            with tc.tile_pool(name="probs", bufs=3) as probs, \
                 tc.tile_pool(name="xq", bufs=2) as xqp, \
                 tc.tile_pool(name="xs", bufs=2) as xsp, \
                 tc.tile_pool(name="small", bufs=2) as small, \
                 tc.tile_pool(name="ost", bufs=4) as ostp, \
                 tc.tile_pool(name="psc", bufs=2, space="PSUM") as psc, \
                 tc.tile_pool(name="pxt", bufs=1, space="PSUM") as pxt, \
                 tc.tile_pool(name="shpp", bufs=2, space="PSUM") as shp:
                for step in _qproj_steps(nc, tc, shp, wq_sb, qres_mv, qsb,
                                         bqk_sb, add_qk_bias, 0, rn):
                    step()
                vsteps = [_vproj_step(nc, shp, vres_st, wv_sb, vsb, tt, rn)
                          for tt in range(NT)]
                vsteps[0]()
                vsteps[1]()
                xq_of = {}
                for qq in range(QC):
                    xq = [xqp.tile([P, QW], BF16, tag=f"x{t}",
                                   name=f"xq{t}_{qq}_{rn}") for t in range(2)]
                    xq_of[qq] = xq
                    if qq == 0:
                        inj0 = vsteps[2:]
                    else:
                        inj0 = (_qproj_steps(nc, tc, shp, wq_sb, qres_mv,
                                             qsb, bqk_sb, add_qk_bias,
                                             qq + 1, rn)
                                if qq + 1 < QC else
                                _oproj_steps(nc, shp, ostp, wp_sb,
                                             xq_of[qq - 1], out, qq - 1, rn,
                                             tc=tc))
                    inj1 = (_qproj_steps(nc, tc, shp, wq_sb, qres_mv, qsb,
                                         bqk_sb, add_qk_bias, 1, rn)
                            if qq == 0 else
                            (_oproj_steps(nc, shp, ostp, wp_sb,
                                          xq_of[qq - 1], out, qq - 1, rn,
                                          tc=tc)
                             if qq < QC - 1 else []))
                    for hp in range(HPG // 2):
                        inj = inj0 if hp == 0 else inj1
                        pt = hp
                        xtA = pxt.tile([P, QW], FP32, tag="xtA")
                        xtB = pxt.tile([P, QW], FP32, tag="xtB")
                        for m in range(NT):
                            sc = psc.tile([P, 2 * QW], FP32, tag="sc")
                            pr = probs.tile([P, 2 * QW], BF16, tag="pr")
                            for j, off in ((0, 0), (1, DH)):
                                nc.tensor.matmul(
                                    sc[:, j * QW:(j + 1) * QW],
                                    ksb[pt][off:off + DH, m * P:(m + 1) * P],
                                    qsb[pt][qq][off:off + DH, :],
                                    start=True, stop=True,
                                    tile_position=(off, 0))
                            nc.scalar.activation(pr[:], sc[:], AF.Exp,
                                                 scale=scale)
                            for j, xt, h in ((0, xtA, 2 * hp),
                                             (1, xtB, 2 * hp + 1)):
                                nc.tensor.matmul(
                                    xt[0:DH + 1, :],
                                    vsb[m][:, h * (DH + 1):(h + 1) * (DH + 1)],
                                    pr[:, j * QW:(j + 1) * QW],
                                    start=(m == 0), stop=(m == NT - 1))
                            if m < len(inj):
                                inj[m]()
                        for step in inj[NT:]:
                            step()
                        # normalization on Vector+GpSimd only; emission order
                        # releases each xt psum bank as fast as possible.
                        den2 = small.tile([1, 2 * QW], FP32, tag="den2")
                        xs2 = xsp.tile([DH, 2 * QW], FP32, tag="xs2",
                                       name=f"xs2_{qq}_{hp}_{rn}")
                        for xt, j in ((xtA, 0), (xtB, 1)):
                            nc.vector.tensor_copy(
                                den2[:, j * QW:(j + 1) * QW],
                                xt[DH:DH + 1, :])
                            nc.vector.tensor_copy(
                                xs2[:, j * QW:(j + 1) * QW], xt[0:DH, :])
                        rde = small.tile([1, 2 * QW], FP32, tag="rde")
                        nc.vector.reciprocal_approx_fast(out=rde[:],
                                                         in_=den2[:])
                        bcg = small.tile([DH, 2 * QW], FP32, tag="bcg")
                        nc.gpsimd.partition_broadcast(bcg[:], rde[:],
                                                      channels=DH)
                        for j, off in ((0, 0), (1, DH)):
                            nc.vector.tensor_mul(
                                xq[pt][off:off + DH, :],
                                xs2[:, j * QW:(j + 1) * QW],
                                bcg[:, j * QW:(j + 1) * QW])
                # tail: output projection for the last q-chunk
                for step in _oproj_steps(nc, shp, ostp, wp_sb,
                                         xq_of[QC - 1], out, QC - 1, rn):
                    step()

    nc.compile()
    return nc


def _vproj_step(nc, pool, vres_st, wv_sb, vsb, tt, rn):
    """V-projection of token-tile tt: 8 accumulating matmuls + the
    psum->sbuf copy + the ones-column memset, as one closure."""
    def step():
        pvt = pool.tile([P, QW], FP32, tag="sh", name=f"pv{tt}_{rn}")
        for k in range(KT):
            nc.tensor.matmul(pvt[:, 0:CS], vres_st(k, tt),
                             wv_sb[:, k * CS:(k + 1) * CS],
                             start=(k == 0), stop=(k == KT - 1))
        dst3 = vsb[tt][:].rearrange("p (h c) -> p h c", h=HPG)
        nc.vector.tensor_copy(
            dst3[:, :, 0:DH],
            pvt[:, 0:CS].rearrange("p (h c) -> p h c", h=HPG))
        nc.vector.memset(dst3[:, :, DH:DH + 1].bitcast(mybir.dt.uint16),
                         0x3f80)
    return step


def _qproj_steps(nc, tc, pool, wq_sb, qres_mv, qsb, bqk_sb, add_qk_bias,
                 c, rn):
    """Q-projection of chunk c as a list of closures, ONE matmul per
    attention m-step (16 steps), then the psum->sbuf copies."""
    state = {}

    def mk_mm(k, t):
        def step():
            if k == 0 and t == 0:
                state["pA"] = pool.tile([P, QW], FP32, tag="sh",
                                        name=f"pjA{c}_{rn}")
                state["pB"] = pool.tile([P, QW], FP32, tag="sh",
                                        name=f"pjB{c}_{rn}")
            dst = state["pA"] if t == 0 else state["pB"]
            w = (wq_sb[:, k * CS:k * CS + P] if t == 0
                 else wq_sb[:, k * CS + P:(k + 1) * CS])
            nc.tensor.matmul(dst[:], w, qres_mv(c, k),
                             start=(k == 0), stop=(k == KT - 1))
        return step

    def mk_copy(t):
        def step():
            pt = state["pA"] if t == 0 else state["pB"]
            with tc.high_priority():
                if add_qk_bias:
                    nc.vector.tensor_scalar(qsb[t][c][:], pt[:],
                                            bqk_sb[:, t:t + 1], None,
                                            mybir.AluOpType.add)
                else:
                    nc.vector.tensor_copy(qsb[t][c][:], pt[:])
        return step

    steps = []
    for k in range(KT):
        steps.append(mk_mm(k, 0))
        steps.append(mk_mm(k, 1))
    return steps + [mk_copy(0), mk_copy(1)]


def _oproj_steps(nc, pool, ostp, wp_sb, xq, out, qq, rn, tc=None):
    """Output projection of chunk qq as per-m-step closures: one mo-tile
    (two accumulating matmuls + copy + DMA out) every two steps."""
    qs = slice(qq * QW, (qq + 1) * QW)
    state = {}
    steps = []

    from contextlib import nullcontext

    def delay():
        return tc.high_priority(offset=-300) if tc is not None else nullcontext()

    def mk_mm(mo, k2):
        def step():
            with delay():
                if k2 == 0:
                    state[mo] = pool.tile([P, QW], FP32, tag="sh",
                                          name=f"po{mo}_{qq}_{rn}")
                nc.tensor.matmul(
                    state[mo][:],
                    wp_sb[:, k2 * DIM + mo * P:k2 * DIM + (mo + 1) * P],
                    xq[k2][:],
                    start=(k2 == 0), stop=(k2 == CS // P - 1))
        return step

    def mk_out(mo):
        def step():
            with delay():
                ost = ostp.tile([P, QW], BF16, tag="ost",
                                name=f"ost{mo}_{qq}_{rn}")
                nc.vector.tensor_copy(ost[:], state[mo][:])
                nc.sync.dma_start(out=out[mo * P:(mo + 1) * P, qs],
                                  in_=ost[:])
        return step

    for mo in range(KT):
        steps.append(mk_mm(mo, 0))

        def both(mo=mo):
            mk_mm(mo, 1)()
            mk_out(mo)()
        steps.append(both)
    return steps


_CACHE = {}


def _get_program(scale: float, add_qk_bias: bool, reps: int = 1,
                 loop_reps=None):
    key = (scale, add_qk_bias, reps, loop_reps)
    if key not in _CACHE:
        _CACHE[key] = _build(scale, add_qk_bias, reps, loop_reps)
    return _CACHE[key]


def _ktile(x, kt):
    """[kt*P, F] -> [P, kt*F] with k-tile blocks along the free dim."""
    f = x.shape[1]
    return np.ascontiguousarray(
        x.reshape(kt, P, f).transpose(1, 0, 2).reshape(P, kt * f))


def make_in_maps(query, key, value, Wq, bq, Wk, bk, Wv, bv, Wp, bp, scale):
    query = np.asarray(query, np.float32)
    key = np.asarray(key, np.float32)
    value = np.asarray(value, np.float32)
    Wq, Wk, Wv, Wp = (np.asarray(a, np.float32) for a in (Wq, Wk, Wv, Wp))
    bq, bk = np.asarray(bq, np.float32), np.asarray(bk, np.float32)
    in_maps = []
    for c in range(NCORES):
        b, g = c // HG, c % HG
        cs = slice(g * CS, (g + 1) * CS)
        bqk_arr = np.stack([bq[cs].reshape(CS // P, P),
                            bk[cs].reshape(CS // P, P)]).reshape(-1, P).T
        # qT chunk-blocked: [P, (c k n)] with c = q-chunk
        qt = query[b].T.reshape(KT, P, QC, QW).transpose(1, 2, 0, 3)
        in_maps.append({
            "qT": np.ascontiguousarray(qt.reshape(P, QC * KT * QW)
                                       ).astype(BF16NP),
            "kT": _ktile(key[b].T, KT).astype(BF16NP),
            "vT": _ktile(value[b].T, KT).astype(BF16NP),
            "wq": _ktile(np.ascontiguousarray(Wq[cs, :].T), KT).astype(BF16NP),
            "wk": _ktile(np.ascontiguousarray(Wk[cs, :].T), KT).astype(BF16NP),
            "wv": _ktile(np.ascontiguousarray(Wv[cs, :].T), KT).astype(BF16NP),
            "wp": _ktile(np.ascontiguousarray(Wp[:, cs].T),
                         CS // P).astype(BF16NP),
            "bqk": np.ascontiguousarray(bqk_arr),
        })
    return in_maps


def combine_outputs(results, bv, bp, Wp):
    bv = np.asarray(bv, np.float32)
    bp = np.asarray(bp, np.float32)
    Wp = np.asarray(Wp, np.float32)
    out = np.empty((B, N, DIM), np.float32)
    corr = bp + bv @ Wp.T
    for b in range(B):
        acc = results[b * HG]["out"].astype(np.float32)
        for g in range(1, HG):
            acc += results[b * HG + g]["out"].astype(np.float32)
        out[b] = acc.T + corr
    return out


def kernel(query, key, value, Wq, bq, Wk, bk, Wv, bv, Wp, bp, scale):
    scale_v = float(np.asarray(scale).reshape(-1)[0])
    add_qk_bias = bool(np.any(np.asarray(bq)) or np.any(np.asarray(bk)))
    nc = _get_program(scale_v, add_qk_bias)
    in_maps = make_in_maps(query, key, value, Wq, bq, Wk, bk, Wv, bv,
                           Wp, bp, scale)
    res = run_bass_kernel_spmd(nc, in_maps, list(range(NCORES))).results
    return combine_outputs(res, bv, bp, Wp)
